# revision 10
# baseline (speedup 1.0000x reference)
"""Tensor-parallel LlamaAttention (GQA + RoPE + causal) for 8 trn2 NeuronCores.

v2 design (faster than the AllGather baseline):
  - x is transposed on the host (xT [H, BT] bf16), so q/k/v projections
    stream xT tiles straight from DRAM -- no on-chip PE transposes.
  - Per-core column-parallel q/k/v by head (NHC q-heads, 1 kv-head per
    core), attention computed locally per head in transposed layout.
  - Softmax row-sums L via DVE bf16 accumulation of P tiles + a single
    M=1 ones-matmul per (b,qb,h); reciprocal on [1,QB] only; broadcast
    back to 128 partitions with a 1-contraction matmul.
  - exp is applied on [128, 2*QB] PSUM spans (one ACT instruction per
    two k-blocks) to amortize the ~352-cycle ACT fixed overhead.
  - AllToAll (token redistribution) instead of AllGather: each core ends
    with the full attention dim for its own token chunk per batch, so
    o_proj is row-parallel over tokens with the full wo streamed from
    DRAM. One A2A per batch, overlapped with attention of the next batch
    and with o_proj.

Layouts:
  xT[h, t] host-transposed; qT/kT[d, t] via projection matmuls;
  vbig[t%128, (t//128)*128+d] natural v tiles packed along free dim;
  S^T[k, q] = matmul(kT slice, qT slice); P^T = exp(scale*S^T);
  O^T[d, q] += matmul(v tile, P^T);  attnT = O^T * (1/L) broadcast.
"""

import math
import sys

import numpy as np

sys.path.insert(0, "/opt/trn_rl_repo")

import ml_dtypes  # noqa: E402

from concourse import bacc, mybir, tile  # noqa: E402
from concourse.bass_utils import run_bass_kernel_spmd  # noqa: E402

F32 = mybir.dt.float32
BF16 = mybir.dt.bfloat16
NCORES = 8
P = 128  # partitions / head dim
QB = 512  # q-block (PSUM free dim)
KB = 128  # k-block (contraction tile)

_CACHE = {}


def build_program(B, S, H, NH, NKV):
    nc = bacc.Bacc("TRN2", num_devices=NCORES)

    BT = B * S
    NHC = NH // NCORES  # q heads per core
    assert NKV // NCORES == 1
    DQ = NHC * P  # per-core attn width
    HB = H // P  # h blocks
    TB = BT // QB  # token super-blocks
    QBB = S // QB  # q blocks per batch
    SC = S // NCORES  # tokens per (core, batch) after A2A
    CPQ = QB // SC  # a2a chunks per q-block
    OCB = H // QB  # o_proj output column chunks
    TT = (SC + P - 1) // P  # token tiles per (core, batch) in o_proj
    scale = 1.0 / math.sqrt(P)

    xT = nc.declare_dram_parameter("xT", [H, BT], BF16, isOutput=False)
    wq_c = nc.declare_dram_parameter("wq_c", [H, DQ], BF16, isOutput=False)
    wk_c = nc.declare_dram_parameter("wk_c", [H, P], BF16, isOutput=False)
    wv_c = nc.declare_dram_parameter("wv_c", [H, P], BF16, isOutput=False)
    wo_full = nc.declare_dram_parameter("wo_full", [NH * P, H], BF16, isOutput=False)
    cos_t = nc.declare_dram_parameter("cos_t", [P, BT], F32, isOutput=False)
    sinx_t = nc.declare_dram_parameter("sinx_t", [P, BT], F32, isOutput=False)
    masks2_t = nc.declare_dram_parameter("masks2_t", [2, P, 2 * QB], BF16, isOutput=False)
    ones_t = nc.declare_dram_parameter("ones_t", [P, P], BF16, isOutput=False)
    y_c = nc.declare_dram_parameter("y_c", [B * SC, H], F32, isOutput=True)

    with tile.TileContext(nc) as tc:
        with (
            tc.tile_pool(name="dram", bufs=1, space="DRAM") as dram,
            tc.tile_pool(name="const", bufs=1) as constp,
            tc.tile_pool(name="persist", bufs=1) as persist,
        ):
            a2a_in = [
                dram.tile([NCORES * DQ, SC], BF16, tag=f"a2ain{b}", name=f"a2ain{b}")
                for b in range(B)
            ]
            a2a_out = [
                dram.tile([NCORES * DQ, SC], BF16, tag=f"a2aout{b}", name=f"a2aout{b}")
                for b in range(B)
            ]

            ones_sb = constp.tile([P, P], BF16, tag="ones")
            nc.sync.dma_start(out=ones_sb, in_=ones_t[:, :])
            mask_sb = [
                constp.tile([P, 2 * QB], BF16, tag=f"mask{o}", name=f"mask{o}")
                for o in range(2)
            ]
            for o in range(2):
                nc.sync.dma_start(out=mask_sb[o], in_=masks2_t[o])

            qT = [
                persist.tile([P, BT], BF16, tag=f"qT{i}", name=f"qT{i}")
                for i in range(NHC)
            ]
            kT = persist.tile([P, BT], BF16, tag="kT")
            vbig = persist.tile([P, BT], BF16, tag="vbig")

            # ---------------- phase 1: q/k/v projections + rope
            with (
                tc.tile_pool(name="xin", bufs=48) as xin_p,
                tc.tile_pool(name="wqkv", bufs=1) as w_p,
                tc.tile_pool(name="tabs", bufs=2) as tab_p,
                tc.tile_pool(name="ropetmp", bufs=4) as rt_p,
                tc.tile_pool(name="psq", bufs=1, space="PSUM") as psq_p,
                tc.tile_pool(name="psk", bufs=2, space="PSUM") as psk_p,
                tc.tile_pool(name="psv", bufs=2, space="PSUM") as psv_p,
            ):
                wq_sb = [w_p.tile([P, DQ], BF16, tag=f"wq{i}", name=f"wq{i}") for i in range(HB)]
                wk_sb = [w_p.tile([P, P], BF16, tag=f"wk{i}", name=f"wk{i}") for i in range(HB)]
                wv_sb = [w_p.tile([P, P], BF16, tag=f"wv{i}", name=f"wv{i}") for i in range(HB)]
                for hb in range(HB):
                    nc.sync.dma_start(out=wq_sb[hb], in_=wq_c[hb * P : (hb + 1) * P, :])
                    nc.sync.dma_start(out=wk_sb[hb], in_=wk_c[hb * P : (hb + 1) * P, :])
                    nc.sync.dma_start(out=wv_sb[hb], in_=wv_c[hb * P : (hb + 1) * P, :])

                def rope(dst, ps, cos_sb, sinx_sb):
                    """dst = ps*cos + shift64(ps)*sinx (all [128,QB])"""
                    t1 = rt_p.tile([P, QB], F32, tag="ropet1")
                    t2 = rt_p.tile([P, QB], F32, tag="ropet2")
                    nc.vector.tensor_tensor(t1, ps, cos_sb, mybir.AluOpType.mult)
                    h = P // 2
                    nc.vector.tensor_tensor(
                        t2[0:h], ps[h:P], sinx_sb[0:h], mybir.AluOpType.mult
                    )
                    nc.vector.tensor_tensor(
                        t2[h:P], ps[0:h], sinx_sb[h:P], mybir.AluOpType.mult
                    )
                    nc.vector.tensor_tensor(dst, t1, t2, mybir.AluOpType.add)

                # q-head interleave groups (pairs when possible)
                dqgroups = [
                    tuple(range(g, min(g + 2, NHC))) for g in range(0, NHC, 2)
                ]

                for tb in range(TB):
                    t0 = tb * QB
                    cos_sb = tab_p.tile([P, QB], F32, tag="cos")
                    sinx_sb = tab_p.tile([P, QB], F32, tag="sinx")
                    nc.sync.dma_start(out=cos_sb, in_=cos_t[:, t0 : t0 + QB])
                    nc.sync.dma_start(out=sinx_sb, in_=sinx_t[:, t0 : t0 + QB])

                    xts = []
                    for hb in range(HB):
                        xi = xin_p.tile([P, QB], BF16, tag="xin")
                        nc.sync.dma_start(
                            out=xi, in_=xT[hb * P : (hb + 1) * P, t0 : t0 + QB]
                        )
                        xts.append(xi)

                    # chain groups: pairs of q-head chains (distinct PSUM banks)
                    # then the k chain; one v column-chain rides along with each
                    # group (v chains share one bank so they must be sequential,
                    # but each can interleave with the q/k chains in other banks)
                    v_ps = psv_p.tile([P, QB], F32, tag="vps")
                    vchains = list(range(QB // P))  # 4 column-chains
                    groups = [("q", grp) for grp in dqgroups] + [("k", None)]
                    while len(groups) < len(vchains):
                        groups.append(("", None))
                    for gi, (kind, grp) in enumerate(groups):
                        vi = vchains[gi] if gi < len(vchains) else None
                        q_ps = {}
                        if kind == "q":
                            q_ps = {
                                dq: psq_p.tile(
                                    [P, QB], F32, tag=f"qps{dq}", name=f"qps{dq}",
                                    bufs=(2 if NHC == 1 else 1),
                                )
                                for dq in grp
                            }
                        elif kind == "k":
                            k_ps = psk_p.tile([P, QB], F32, tag="kps")
                        for hb in range(HB):
                            if kind == "q":
                                for dq in grp:
                                    nc.tensor.matmul(
                                        q_ps[dq],
                                        wq_sb[hb][:, dq * P : (dq + 1) * P],
                                        xts[hb],
                                        start=(hb == 0),
                                        stop=(hb == HB - 1),
                                    )
                            elif kind == "k":
                                nc.tensor.matmul(
                                    k_ps,
                                    wk_sb[hb],
                                    xts[hb],
                                    start=(hb == 0),
                                    stop=(hb == HB - 1),
                                )
                            if vi is not None:
                                nc.tensor.matmul(
                                    v_ps[:, vi * P : (vi + 1) * P],
                                    xts[hb][:, vi * P : (vi + 1) * P],
                                    wv_sb[hb],
                                    start=(hb == 0),
                                    stop=(hb == HB - 1),
                                )
                        if kind == "q":
                            for dq in grp:
                                rope(qT[dq][:, t0 : t0 + QB], q_ps[dq], cos_sb, sinx_sb)
                        elif kind == "k":
                            rope(kT[:, t0 : t0 + QB], k_ps, cos_sb, sinx_sb)
                    nc.vector.tensor_copy(vbig[:, t0 : t0 + QB], v_ps)

            # ---------------- phase 2: attention + per-batch AllToAll
            with (
                tc.tile_pool(name="pP", bufs=3) as p_p,
                tc.tile_pool(name="aout", bufs=4) as ao_p,
                tc.tile_pool(name="rv", bufs=2) as rv_p,
                tc.tile_pool(name="psS", bufs=2, space="PSUM") as pss_p,
                tc.tile_pool(name="psO", bufs=2, space="PSUM") as pso_p,
                tc.tile_pool(name="psL", bufs=1, space="PSUM") as psl_p,
                tc.tile_pool(name="psR", bufs=1, space="PSUM") as psr_p,
            ):
                for b in range(B):
                    for qb in range(QBB):
                        for h in range(NHC):
                            q0 = b * S + qb * QB
                            npair = (qb + 1) * (QB // KB) // 2
                            o_ps = pso_p.tile([P, QB], F32, tag="ops")
                            l_ps = psl_p.tile([1, QB], F32, tag="lps")
                            for j in range(npair):
                                k0 = b * S + j * 2 * KB
                                s_ps = pss_p.tile([P, 2 * QB], F32, tag="sps")
                                nc.tensor.matmul(
                                    s_ps[:, 0:QB],
                                    kT[:, k0 : k0 + KB],
                                    qT[h][:, q0 : q0 + QB],
                                    start=True,
                                    stop=True,
                                )
                                nc.tensor.matmul(
                                    s_ps[:, QB:],
                                    kT[:, k0 + KB : k0 + 2 * KB],
                                    qT[h][:, q0 : q0 + QB],
                                    start=True,
                                    stop=True,
                                )
                                p_sb = p_p.tile([P, 2 * QB], BF16, tag="P")
                                nc.scalar.activation(
                                    p_sb,
                                    s_ps,
                                    mybir.ActivationFunctionType.Exp,
                                    scale=scale,
                                )
                                jd = j - qb * (QB // KB) // 2
                                if jd >= 0:
                                    nc.vector.tensor_tensor(
                                        p_sb, p_sb, mask_sb[jd], mybir.AluOpType.mult
                                    )
                                nc.tensor.matmul(
                                    o_ps,
                                    vbig[:, k0 : k0 + P],
                                    p_sb[:, 0:QB],
                                    start=(j == 0),
                                    stop=False,
                                )
                                nc.tensor.matmul(
                                    o_ps,
                                    vbig[:, k0 + P : k0 + 2 * P],
                                    p_sb[:, QB:],
                                    start=False,
                                    stop=(j == npair - 1),
                                )
                                nc.tensor.matmul(
                                    l_ps,
                                    ones_sb[:, 0:1],
                                    p_sb[:, 0:QB],
                                    start=(j == 0),
                                    stop=False,
                                )
                                nc.tensor.matmul(
                                    l_ps,
                                    ones_sb[:, 0:1],
                                    p_sb[:, QB:],
                                    start=False,
                                    stop=(j == npair - 1),
                                )
                            rinv = rv_p.tile([1, QB], F32, tag="rinv")
                            nc.vector.reciprocal_approx_fast(out=rinv, in_=l_ps)
                            rinv_b = rv_p.tile([1, QB], BF16, tag="rinvb")
                            nc.vector.tensor_copy(rinv_b, rinv)
                            rb_ps = psr_p.tile([P, QB], F32, tag="rbps")
                            nc.tensor.matmul(
                                rb_ps, ones_sb[0:1, :], rinv_b, start=True, stop=True
                            )
                            rb_sb = rv_p.tile([P, QB], F32, tag="rbsb")
                            nc.vector.tensor_copy(rb_sb, rb_ps)
                            attn_sb = ao_p.tile([P, QB], BF16, tag="attn")
                            nc.vector.tensor_tensor(
                                attn_sb, o_ps, rb_sb, mybir.AluOpType.mult
                            )
                            for cc in range(CPQ):
                                ch = qb * CPQ + cc
                                nc.sync.dma_start(
                                    out=a2a_in[b][
                                        ch * DQ + h * P : ch * DQ + (h + 1) * P, :
                                    ],
                                    in_=attn_sb[:, cc * SC : (cc + 1) * SC],
                                )
                    nc.gpsimd.collective_compute(
                        "AllToAll",
                        mybir.AluOpType.bypass,
                        replica_groups=[list(range(NCORES))],
                        ins=[a2a_in[b][:, :]],
                        outs=[a2a_out[b][:, :]],
                    )

            # ---------------- phase 3: row-parallel o_proj on own token chunks
            with (
                tc.tile_pool(name="aT", bufs=1) as at_p,
                tc.tile_pool(name="wo", bufs=2 * HB) as wo_p,
                tc.tile_pool(name="yout", bufs=4) as y_p,
                tc.tile_pool(name="psY", bufs=1, space="PSUM") as psy_p,
            ):
                at_sb = []
                for b in range(B):
                    row = []
                    for dblk in range(HB):
                        a = at_p.tile([P, SC], BF16, tag=f"aT{b}_{dblk}", name=f"aT{b}_{dblk}")
                        nc.sync.dma_start(
                            out=a, in_=a2a_out[b][dblk * P : (dblk + 1) * P, :]
                        )
                        row.append(a)
                    at_sb.append(row)

                for oc in range(OCB):
                    wo_sb = []
                    for dblk in range(HB):
                        w = wo_p.tile([P, QB], BF16, tag="wo")
                        nc.sync.dma_start(
                            out=w,
                            in_=wo_full[dblk * P : (dblk + 1) * P, oc * QB : (oc + 1) * QB],
                        )
                        wo_sb.append(w)
                    for b in range(B):
                        tts = [
                            (tt * P, min(SC - tt * P, P)) for tt in range(TT)
                        ]
                        y_ps = [
                            psy_p.tile([tw, QB], F32, tag=f"yps{b}_{i}", name=f"yps{b}_{i}")
                            for i, (toff, tw) in enumerate(tts)
                        ]
                        for dblk in range(HB):
                            for i, (toff, tw) in enumerate(tts):
                                nc.tensor.matmul(
                                    y_ps[i],
                                    at_sb[b][dblk][:, toff : toff + tw],
                                    wo_sb[dblk],
                                    start=(dblk == 0),
                                    stop=(dblk == HB - 1),
                                )
                        for i, (toff, tw) in enumerate(tts):
                            y_sb = y_p.tile([tw, QB], F32, tag="ysb")
                            if i % 2 == 0:
                                nc.scalar.copy(y_sb, y_ps[i])
                            else:
                                nc.vector.tensor_copy(y_sb, y_ps[i])
                            nc.sync.dma_start(
                                out=y_c[
                                    b * SC + toff : b * SC + toff + tw,
                                    oc * QB : (oc + 1) * QB,
                                ],
                                in_=y_sb,
                            )
    nc.finalize()
    return nc


def _prep_inputs(hidden_states, wq, wk, wv, wo, position_ids, B, S, H, NH, NKV):
    """Host-side: bf16 casts, x transpose, rope tables, paired causal masks,
    per-core weight slices."""
    BT = B * S
    NHC = NH // NCORES
    DQ = NHC * P

    bf = ml_dtypes.bfloat16
    x2d = np.asarray(hidden_states, dtype=np.float32).reshape(BT, H)
    xT = np.ascontiguousarray(x2d.T).astype(bf)
    wq_b, wk_b, wv_b, wo_b = (np.asarray(w).astype(bf) for w in (wq, wk, wv, wo))

    half = P // 2
    inv_freq = 1.0 / (10000.0 ** (np.arange(half, dtype=np.float64) / half))
    pos = np.asarray(position_ids).astype(np.float64).reshape(BT)
    ang = pos[None, :] * inv_freq[:, None]  # [64, BT]
    cos_t = np.concatenate([np.cos(ang), np.cos(ang)], 0).astype(np.float32)
    sinx_t = np.concatenate([-np.sin(ang), np.sin(ang)], 0).astype(np.float32)

    # paired diagonal-block causal masks over [128 k, 2*QB (2 k-blocks x q)]
    kk = np.arange(KB)[None, :, None]
    qq = np.arange(QB)[None, None, :]
    masks2 = np.zeros((2, P, 2 * QB), dtype=bf)
    for jd in range(2):
        for hhalf in range(2):
            krel = (2 * jd + hhalf) * KB + kk
            masks2[jd : jd + 1, :, hhalf * QB : (hhalf + 1) * QB] = (krel <= qq).astype(bf)
    ones_m = np.ones((P, P), dtype=bf)

    in_maps = []
    for c in range(NCORES):
        in_maps.append(
            {
                "xT": xT,
                "wq_c": np.ascontiguousarray(wq_b[:, c * DQ : (c + 1) * DQ]),
                "wk_c": np.ascontiguousarray(wk_b[:, c * P : (c + 1) * P]),
                "wv_c": np.ascontiguousarray(wv_b[:, c * P : (c + 1) * P]),
                "wo_full": wo_b,
                "cos_t": cos_t,
                "sinx_t": sinx_t,
                "masks2_t": masks2,
                "ones_t": ones_m,
            }
        )
    return in_maps


def run(hidden_states, wq, wk, wv, wo, position_ids, B, S, H, NH, NKV, trace=False):
    key = (B, S, H, NH, NKV)
    if key not in _CACHE:
        _CACHE[key] = build_program(B, S, H, NH, NKV)
    nc = _CACHE[key]
    in_maps = _prep_inputs(
        hidden_states, wq, wk, wv, wo, position_ids, B, S, H, NH, NKV
    )
    res = run_bass_kernel_spmd(nc, in_maps, core_ids=list(range(NCORES)), trace=trace)
    SC = S // NCORES
    out = np.empty((B, S, NH * P), dtype=np.float32)
    for c in range(NCORES):
        yc = res.results[c]["y_c"]
        for b in range(B):
            out[b, c * SC : (c + 1) * SC, :] = yc[b * SC : (b + 1) * SC, :]
    return (out, res) if trace else (out, None)


def kernel(hidden_states, wq, wk, wv, wo, position_ids):
    out, _ = run(
        hidden_states, wq, wk, wv, wo, position_ids, 2, 2048, 4096, 32, 8
    )
    return out


# revision 12
# speedup vs baseline: 1.0840x; 1.0840x over previous
"""Tensor-parallel LlamaAttention (GQA + RoPE + causal) for 8 trn2 NeuronCores.

v4 design:
  - x transposed on host (xT [H, BT] bf16): projections stream xT straight
    from DRAM, no on-chip transposes.
  - Column-parallel q/k/v by head (NHC q-heads, 1 kv-head per core);
    attention in transposed layout, paired k-blocks: S^T on [128, 2*QB]
    PSUM spans, one exp ACT instruction per pair.
  - Softmax denominator: DVE bf16 accumulation T += P per pair, then two
    M=1 ones-matmuls into l_ps[1,QB]; reciprocal_approx_fast on [1,QB];
    broadcast to 128 partitions via a 1-contraction matmul.
  - Normalization tail is software-pipelined: deferred into the next
    (b,qb,h) iteration so the PE stream never stalls on the DVE chain.
  - Four split AllToAlls (one per half-batch) redistribute attention
    output by token; each overlaps the next attention chunk / o_proj.
  - o_proj is row-parallel over tokens with full wo streamed from DRAM;
    the first ocs' wo tiles are prefetched during phase 2.
"""

import math
import sys

import numpy as np

sys.path.insert(0, "/opt/trn_rl_repo")

import ml_dtypes  # noqa: E402

from concourse import bacc, mybir, tile  # noqa: E402
from concourse.bass_utils import run_bass_kernel_spmd  # noqa: E402

F32 = mybir.dt.float32
BF16 = mybir.dt.bfloat16
NCORES = 8
P = 128  # partitions / head dim
QB = 512  # q-block (PSUM free dim)
KB = 128  # k-block (contraction tile)

_CACHE = {}


def build_program(B, S, H, NH, NKV):
    nc = bacc.Bacc("TRN2", num_devices=NCORES)

    BT = B * S
    NHC = NH // NCORES  # q heads per core
    assert NKV // NCORES == 1
    assert NH * P == H
    DQ = NHC * P  # per-core attn width
    HB = H // P  # h blocks
    TB = BT // QB  # token super-blocks
    QBB = S // QB  # q blocks per batch
    GQA = min(2, QBB)  # q-blocks per AllToAll
    NQA = (QBB + GQA - 1) // GQA  # AllToAlls per batch
    NA = B * NQA  # total AllToAlls
    SCA = GQA * QB // NCORES  # tokens per (core, a2a)
    CPQ = QB // SCA  # a2a chunks per q-block
    OCB = H // QB  # o_proj output column chunks
    WOPRE = min(3, OCB)  # wo ocs prefetched during phase 2
    ATG = min(8, HB)  # dblks per a2a-out load DMA
    scale = 1.0 / math.sqrt(P)

    xT = nc.declare_dram_parameter("xT", [H, BT], BF16, isOutput=False)
    wq_c = nc.declare_dram_parameter("wq_c", [H, DQ], BF16, isOutput=False)
    wk_c = nc.declare_dram_parameter("wk_c", [H, P], BF16, isOutput=False)
    wv_c = nc.declare_dram_parameter("wv_c", [H, P], BF16, isOutput=False)
    wo_full = nc.declare_dram_parameter("wo_full", [NH * P, H], BF16, isOutput=False)
    cos_t = nc.declare_dram_parameter("cos_t", [P, BT], F32, isOutput=False)
    sinx_t = nc.declare_dram_parameter("sinx_t", [P, BT], F32, isOutput=False)
    masks2_t = nc.declare_dram_parameter("masks2_t", [2, P, 2 * QB], BF16, isOutput=False)
    ones_t = nc.declare_dram_parameter("ones_t", [P, P], BF16, isOutput=False)
    y_c = nc.declare_dram_parameter("y_c", [NA * SCA, H], F32, isOutput=True)

    with tile.TileContext(nc) as tc:
        with (
            tc.tile_pool(name="dram", bufs=1, space="DRAM") as dram,
            tc.tile_pool(name="const", bufs=1) as constp,
            tc.tile_pool(name="persist", bufs=1) as persist,
        ):
            a2a_in = [
                dram.tile([NCORES * DQ, SCA], BF16, tag=f"a2ain{a}", name=f"a2ain{a}")
                for a in range(NA)
            ]
            a2a_out = [
                dram.tile([NCORES * DQ, SCA], BF16, tag=f"a2aout{a}", name=f"a2aout{a}")
                for a in range(NA)
            ]

            ones_sb = constp.tile([P, P], BF16, tag="ones")
            nc.sync.dma_start(out=ones_sb, in_=ones_t[:, :])
            mask_sb = [
                constp.tile([P, 2 * QB], BF16, tag=f"mask{o}", name=f"mask{o}")
                for o in range(2)
            ]
            for o in range(2):
                nc.sync.dma_start(out=mask_sb[o], in_=masks2_t[o])

            qT = [
                persist.tile([P, BT], BF16, tag=f"qT{i}", name=f"qT{i}")
                for i in range(NHC)
            ]
            kT = persist.tile([P, BT], BF16, tag="kT")
            vbig = persist.tile([P, BT], BF16, tag="vbig")

            # ---------------- phase 1: q/k/v projections + rope
            with (
                tc.tile_pool(name="xin", bufs=48) as xin_p,
                tc.tile_pool(name="wqkv", bufs=1) as w_p,
                tc.tile_pool(name="tabs", bufs=2) as tab_p,
                tc.tile_pool(name="ropetmp", bufs=4) as rt_p,
                tc.tile_pool(name="psq", bufs=1, space="PSUM") as psq_p,
                tc.tile_pool(name="psk", bufs=2, space="PSUM") as psk_p,
                tc.tile_pool(name="psv", bufs=2, space="PSUM") as psv_p,
            ):
                wq_sb = [w_p.tile([P, DQ], BF16, tag=f"wq{i}", name=f"wq{i}") for i in range(HB)]
                wk_sb = [w_p.tile([P, P], BF16, tag=f"wk{i}", name=f"wk{i}") for i in range(HB)]
                wv_sb = [w_p.tile([P, P], BF16, tag=f"wv{i}", name=f"wv{i}") for i in range(HB)]
                for hb in range(HB):
                    nc.sync.dma_start(out=wq_sb[hb], in_=wq_c[hb * P : (hb + 1) * P, :])
                    nc.sync.dma_start(out=wk_sb[hb], in_=wk_c[hb * P : (hb + 1) * P, :])
                    nc.sync.dma_start(out=wv_sb[hb], in_=wv_c[hb * P : (hb + 1) * P, :])

                def rope(dst, ps, cos_sb, sinx_sb):
                    """dst = ps*cos + shift64(ps)*sinx (all [128,QB])"""
                    t1 = rt_p.tile([P, QB], F32, tag="ropet1")
                    t2 = rt_p.tile([P, QB], F32, tag="ropet2")
                    nc.vector.tensor_tensor(t1, ps, cos_sb, mybir.AluOpType.mult)
                    h = P // 2
                    nc.vector.tensor_tensor(
                        t2[0:h], ps[h:P], sinx_sb[0:h], mybir.AluOpType.mult
                    )
                    nc.vector.tensor_tensor(
                        t2[h:P], ps[0:h], sinx_sb[h:P], mybir.AluOpType.mult
                    )
                    nc.vector.tensor_tensor(dst, t1, t2, mybir.AluOpType.add)

                dqgroups = [
                    tuple(range(g, min(g + 2, NHC))) for g in range(0, NHC, 2)
                ]

                for tb in range(TB):
                    t0 = tb * QB
                    cos_sb = tab_p.tile([P, QB], F32, tag="cos")
                    sinx_sb = tab_p.tile([P, QB], F32, tag="sinx")
                    nc.sync.dma_start(out=cos_sb, in_=cos_t[:, t0 : t0 + QB])
                    nc.sync.dma_start(out=sinx_sb, in_=sinx_t[:, t0 : t0 + QB])

                    xts = []
                    for hb in range(HB):
                        xi = xin_p.tile([P, QB], BF16, tag="xin")
                        nc.sync.dma_start(
                            out=xi, in_=xT[hb * P : (hb + 1) * P, t0 : t0 + QB]
                        )
                        xts.append(xi)

                    # chain groups: q-head pairs, then k; one v column-chain per
                    # group (v chains share a bank -> sequential among selves)
                    v_ps = psv_p.tile([P, QB], F32, tag="vps")
                    vchains = list(range(QB // P))
                    groups = [("q", grp) for grp in dqgroups] + [("k", None)]
                    while len(groups) < len(vchains):
                        groups.append(("", None))
                    for gi, (kind, grp) in enumerate(groups):
                        vi = vchains[gi] if gi < len(vchains) else None
                        q_ps = {}
                        if kind == "q":
                            q_ps = {
                                dq: psq_p.tile(
                                    [P, QB], F32, tag=f"qps{dq}", name=f"qps{dq}",
                                    bufs=(2 if NHC == 1 else 1),
                                )
                                for dq in grp
                            }
                        elif kind == "k":
                            k_ps = psk_p.tile([P, QB], F32, tag="kps")
                        for hb in range(HB):
                            if kind == "q":
                                for dq in grp:
                                    nc.tensor.matmul(
                                        q_ps[dq],
                                        wq_sb[hb][:, dq * P : (dq + 1) * P],
                                        xts[hb],
                                        start=(hb == 0),
                                        stop=(hb == HB - 1),
                                    )
                            elif kind == "k":
                                nc.tensor.matmul(
                                    k_ps,
                                    wk_sb[hb],
                                    xts[hb],
                                    start=(hb == 0),
                                    stop=(hb == HB - 1),
                                )
                            if vi is not None:
                                nc.tensor.matmul(
                                    v_ps[:, vi * P : (vi + 1) * P],
                                    xts[hb][:, vi * P : (vi + 1) * P],
                                    wv_sb[hb],
                                    start=(hb == 0),
                                    stop=(hb == HB - 1),
                                )
                        if kind == "q":
                            for dq in grp:
                                rope(qT[dq][:, t0 : t0 + QB], q_ps[dq], cos_sb, sinx_sb)
                        elif kind == "k":
                            rope(kT[:, t0 : t0 + QB], k_ps, cos_sb, sinx_sb)
                    nc.vector.tensor_copy(vbig[:, t0 : t0 + QB], v_ps)

            # ---------------- phases 2+3 share the wo pool (prefetch)
            with tc.tile_pool(name="wo", bufs=3 * HB) as wo_p:
                wo_sb = {}  # (oc, dblk) -> tile
                wo_pending = [(oc, dblk) for oc in range(WOPRE) for dblk in range(HB)]
                wo_ptr = [0]

                def emit_wo(n):
                    end = min(wo_ptr[0] + n, len(wo_pending))
                    for idx in range(wo_ptr[0], end):
                        oc, dblk = wo_pending[idx]
                        w = wo_p.tile([P, QB], BF16, tag="wo", name=f"wo{oc}_{dblk}")
                        nc.sync.dma_start(
                            out=w,
                            in_=wo_full[
                                dblk * P : (dblk + 1) * P, oc * QB : (oc + 1) * QB
                            ],
                        )
                        wo_sb[(oc, dblk)] = w
                    wo_ptr[0] = end

                # ---------------- phase 2: attention + split AllToAlls
                with (
                    tc.tile_pool(name="pP", bufs=3) as p_p,
                    tc.tile_pool(name="pT", bufs=2) as t_p,
                    tc.tile_pool(name="aout", bufs=4) as ao_p,
                    tc.tile_pool(name="rv", bufs=2) as rv_p,
                    tc.tile_pool(name="psS", bufs=2, space="PSUM") as pss_p,
                    tc.tile_pool(name="psO", bufs=2, space="PSUM") as pso_p,
                    tc.tile_pool(name="psL", bufs=1, space="PSUM") as psl_p,
                    tc.tile_pool(name="psR", bufs=1, space="PSUM") as psr_p,
                ):
                    def emit_norm(st):
                        """deferred normalization tail of a previous iteration"""
                        o_ps, l_ps, aidx, chunk0, h = st
                        rinv = rv_p.tile([1, QB], F32, tag="rinv")
                        nc.vector.reciprocal_approx_fast(out=rinv, in_=l_ps)
                        rinv_b = rv_p.tile([1, QB], BF16, tag="rinvb")
                        nc.vector.tensor_copy(rinv_b, rinv)
                        rb_ps = psr_p.tile([P, QB], F32, tag="rbps")
                        nc.tensor.matmul(
                            rb_ps, ones_sb[0:1, :], rinv_b, start=True, stop=True
                        )
                        rb_sb = rv_p.tile([P, QB], F32, tag="rbsb")
                        nc.scalar.copy(rb_sb, rb_ps)
                        attn_sb = ao_p.tile([P, QB], BF16, tag="attn")
                        nc.vector.tensor_tensor(
                            attn_sb, o_ps, rb_sb, mybir.AluOpType.mult
                        )
                        for cc in range(CPQ):
                            nc.sync.dma_start(
                                out=a2a_in[aidx][
                                    (chunk0 + cc) * DQ + h * P :
                                    (chunk0 + cc) * DQ + (h + 1) * P,
                                    :,
                                ],
                                in_=attn_sb[:, cc * SCA : (cc + 1) * SCA],
                            )

                    prev = None
                    for b in range(B):
                        for qq in range(NQA):
                            aidx = b * NQA + qq
                            for qb in range(qq * GQA, min((qq + 1) * GQA, QBB)):
                                for h in range(NHC):
                                    q0 = b * S + qb * QB
                                    npair = (qb + 1) * (QB // KB) // 2
                                    o_ps = pso_p.tile([P, QB], F32, tag="ops")
                                    t_sb = t_p.tile([P, 2 * QB], BF16, tag="T")
                                    for j in range(npair):
                                        if j == 1 and prev is not None:
                                            emit_norm(prev)
                                            prev = None
                                        k0 = b * S + j * 2 * KB
                                        s_ps = pss_p.tile([P, 2 * QB], F32, tag="sps")
                                        nc.tensor.matmul(
                                            s_ps[:, 0:QB],
                                            kT[:, k0 : k0 + KB],
                                            qT[h][:, q0 : q0 + QB],
                                            start=True,
                                            stop=True,
                                        )
                                        nc.tensor.matmul(
                                            s_ps[:, QB:],
                                            kT[:, k0 + KB : k0 + 2 * KB],
                                            qT[h][:, q0 : q0 + QB],
                                            start=True,
                                            stop=True,
                                        )
                                        p_sb = p_p.tile([P, 2 * QB], BF16, tag="P")
                                        nc.scalar.activation(
                                            p_sb,
                                            s_ps,
                                            mybir.ActivationFunctionType.Exp,
                                            scale=scale,
                                        )
                                        jd = j - qb * (QB // KB) // 2
                                        if jd >= 0:
                                            nc.vector.tensor_tensor(
                                                p_sb, p_sb, mask_sb[jd],
                                                mybir.AluOpType.mult,
                                            )
                                        if j == 0:
                                            nc.vector.tensor_copy(t_sb, p_sb)
                                        else:
                                            nc.vector.tensor_tensor(
                                                t_sb, t_sb, p_sb, mybir.AluOpType.add
                                            )
                                        nc.tensor.matmul(
                                            o_ps,
                                            vbig[:, k0 : k0 + P],
                                            p_sb[:, 0:QB],
                                            start=(j == 0),
                                            stop=False,
                                        )
                                        nc.tensor.matmul(
                                            o_ps,
                                            vbig[:, k0 + P : k0 + 2 * P],
                                            p_sb[:, QB:],
                                            start=False,
                                            stop=(j == npair - 1),
                                        )
                                    l_ps = psl_p.tile([1, QB], F32, tag="lps")
                                    nc.tensor.matmul(
                                        l_ps, ones_sb[:, 0:1], t_sb[:, 0:QB],
                                        start=True, stop=False,
                                    )
                                    nc.tensor.matmul(
                                        l_ps, ones_sb[:, 0:1], t_sb[:, QB:],
                                        start=False, stop=True,
                                    )
                                    prev = (o_ps, l_ps, aidx, (qb % GQA) * CPQ, h)
                                    emit_wo(3)
                            # flush before the collective so its inputs are emitted
                            if prev is not None:
                                emit_norm(prev)
                                prev = None
                            nc.gpsimd.collective_compute(
                                "AllToAll",
                                mybir.AluOpType.bypass,
                                replica_groups=[list(range(NCORES))],
                                ins=[a2a_in[aidx][:, :]],
                                outs=[a2a_out[aidx][:, :]],
                            )

                # ---------------- phase 3: row-parallel o_proj on own tokens
                with (
                    tc.tile_pool(name="aT", bufs=1) as at_p,
                    tc.tile_pool(name="yout", bufs=4) as y_p,
                    tc.tile_pool(name="psY", bufs=1, space="PSUM") as psy_p,
                ):
                    at_sb = []  # [a][g] tile [P, ATG*SCA]
                    for a in range(NA):
                        row = []
                        for g in range(HB // ATG):
                            t = at_p.tile(
                                [P, ATG * SCA], BF16, tag=f"aT{a}_{g}", name=f"aT{a}_{g}"
                            )
                            src = a2a_out[a][
                                g * ATG * P : (g + 1) * ATG * P, :
                            ].rearrange("(g p) s -> p g s", g=ATG)
                            dst = t.rearrange("p (g s) -> p g s", g=ATG)
                            nc.sync.dma_start(out=dst, in_=src)
                            row.append(t)
                        at_sb.append(row)

                    def at_slice(a, dblk):
                        g, j = dblk // ATG, dblk % ATG
                        return at_sb[a][g][:, j * SCA : (j + 1) * SCA]

                    for oc in range(OCB):
                        emit_wo(HB)  # no-op once pending prefetch is drained
                        if oc >= WOPRE:
                            for dblk in range(HB):
                                w = wo_p.tile([P, QB], BF16, tag="wo", name=f"wo{oc}_{dblk}")
                                nc.sync.dma_start(
                                    out=w,
                                    in_=wo_full[
                                        dblk * P : (dblk + 1) * P,
                                        oc * QB : (oc + 1) * QB,
                                    ],
                                )
                                wo_sb[(oc, dblk)] = w
                        for a0 in range(0, NA, 2):
                            pair = [a for a in (a0, a0 + 1) if a < NA]
                            y_ps = {
                                a: psy_p.tile(
                                    [SCA, QB], F32, tag=f"yps{a}", name=f"yps{a}"
                                )
                                for a in pair
                            }
                            for dblk in range(HB):
                                for a in pair:
                                    nc.tensor.matmul(
                                        y_ps[a],
                                        at_slice(a, dblk),
                                        wo_sb[(oc, dblk)],
                                        start=(dblk == 0),
                                        stop=(dblk == HB - 1),
                                    )
                            for ai, a in enumerate(pair):
                                y_sb = y_p.tile([SCA, QB], F32, tag="ysb")
                                if (a0 + ai) % 2 == 0:
                                    nc.scalar.copy(y_sb, y_ps[a])
                                else:
                                    nc.vector.tensor_copy(y_sb, y_ps[a])
                                nc.sync.dma_start(
                                    out=y_c[
                                        a * SCA : (a + 1) * SCA,
                                        oc * QB : (oc + 1) * QB,
                                    ],
                                    in_=y_sb,
                                )
    nc.finalize()
    return nc


def _prep_inputs(hidden_states, wq, wk, wv, wo, position_ids, B, S, H, NH, NKV):
    """Host-side: bf16 casts, x transpose, rope tables, paired causal masks,
    per-core weight slices."""
    BT = B * S
    NHC = NH // NCORES
    DQ = NHC * P

    bf = ml_dtypes.bfloat16
    x2d = np.asarray(hidden_states, dtype=np.float32).reshape(BT, H)
    xT = np.ascontiguousarray(x2d.T).astype(bf)
    wq_b, wk_b, wv_b, wo_b = (np.asarray(w).astype(bf) for w in (wq, wk, wv, wo))

    half = P // 2
    inv_freq = 1.0 / (10000.0 ** (np.arange(half, dtype=np.float64) / half))
    pos = np.asarray(position_ids).astype(np.float64).reshape(BT)
    ang = pos[None, :] * inv_freq[:, None]  # [64, BT]
    cos_t = np.concatenate([np.cos(ang), np.cos(ang)], 0).astype(np.float32)
    sinx_t = np.concatenate([-np.sin(ang), np.sin(ang)], 0).astype(np.float32)

    # paired diagonal-block causal masks over [128 k, 2*QB (2 k-blocks x q)]
    kk = np.arange(KB)[None, :, None]
    qq = np.arange(QB)[None, None, :]
    masks2 = np.zeros((2, P, 2 * QB), dtype=bf)
    for jd in range(2):
        for hhalf in range(2):
            krel = (2 * jd + hhalf) * KB + kk
            masks2[jd : jd + 1, :, hhalf * QB : (hhalf + 1) * QB] = (krel <= qq).astype(bf)
    ones_m = np.ones((P, P), dtype=bf)

    in_maps = []
    for c in range(NCORES):
        in_maps.append(
            {
                "xT": xT,
                "wq_c": np.ascontiguousarray(wq_b[:, c * DQ : (c + 1) * DQ]),
                "wk_c": np.ascontiguousarray(wk_b[:, c * P : (c + 1) * P]),
                "wv_c": np.ascontiguousarray(wv_b[:, c * P : (c + 1) * P]),
                "wo_full": wo_b,
                "cos_t": cos_t,
                "sinx_t": sinx_t,
                "masks2_t": masks2,
                "ones_t": ones_m,
            }
        )
    return in_maps


def run(hidden_states, wq, wk, wv, wo, position_ids, B, S, H, NH, NKV, trace=False):
    key = (B, S, H, NH, NKV)
    if key not in _CACHE:
        _CACHE[key] = build_program(B, S, H, NH, NKV)
    nc = _CACHE[key]
    in_maps = _prep_inputs(
        hidden_states, wq, wk, wv, wo, position_ids, B, S, H, NH, NKV
    )
    res = run_bass_kernel_spmd(nc, in_maps, core_ids=list(range(NCORES)), trace=trace)
    QBB = S // QB
    GQA = min(2, QBB)
    NQA = (QBB + GQA - 1) // GQA
    NA = B * NQA
    SCA = GQA * QB // NCORES
    out = np.empty((B, S, NH * P), dtype=np.float32)
    for c in range(NCORES):
        yc = res.results[c]["y_c"]
        for a in range(NA):
            b, qq = a // NQA, a % NQA
            tok0 = qq * GQA * QB + c * SCA
            out[b, tok0 : tok0 + SCA, :] = yc[a * SCA : (a + 1) * SCA, :]
    return (out, res) if trace else (out, None)


def kernel(hidden_states, wq, wk, wv, wo, position_ids):
    out, _ = run(
        hidden_states, wq, wk, wv, wo, position_ids, 2, 2048, 4096, 32, 8
    )
    return out


# revision 13
# speedup vs baseline: 1.1206x; 1.0338x over previous
"""Tensor-parallel LlamaAttention (GQA + RoPE + causal) for 8 trn2 NeuronCores.

v4 design:
  - x transposed on host (xT [H, BT] bf16): projections stream xT straight
    from DRAM, no on-chip transposes.
  - Column-parallel q/k/v by head (NHC q-heads, 1 kv-head per core);
    attention in transposed layout, paired k-blocks: S^T on [128, 2*QB]
    PSUM spans, one exp ACT instruction per pair.
  - Softmax denominator: DVE bf16 accumulation T += P per pair, then two
    M=1 ones-matmuls into l_ps[1,QB]; reciprocal_approx_fast on [1,QB];
    broadcast to 128 partitions via a 1-contraction matmul.
  - Normalization tail is software-pipelined: deferred into the next
    (b,qb,h) iteration so the PE stream never stalls on the DVE chain.
  - Four split AllToAlls (one per half-batch) redistribute attention
    output by token; each overlaps the next attention chunk / o_proj.
  - o_proj is row-parallel over tokens with full wo streamed from DRAM;
    the first ocs' wo tiles are prefetched during phase 2.
"""

import math
import sys

import numpy as np

sys.path.insert(0, "/opt/trn_rl_repo")

import ml_dtypes  # noqa: E402

from concourse import bacc, mybir, tile  # noqa: E402
from concourse.bass_utils import run_bass_kernel_spmd  # noqa: E402

F32 = mybir.dt.float32
BF16 = mybir.dt.bfloat16
NCORES = 8
P = 128  # partitions / head dim
QB = 512  # q-block (PSUM free dim)
KB = 128  # k-block (contraction tile)

_CACHE = {}


def build_program(B, S, H, NH, NKV):
    nc = bacc.Bacc("TRN2", num_devices=NCORES)

    BT = B * S
    NHC = NH // NCORES  # q heads per core
    assert NKV // NCORES == 1
    assert NH * P == H
    DQ = NHC * P  # per-core attn width
    HB = H // P  # h blocks
    TB = BT // QB  # token super-blocks
    QBB = S // QB  # q blocks per batch
    GQA = min(2, QBB)  # q-blocks per AllToAll
    NQA = (QBB + GQA - 1) // GQA  # AllToAlls per batch
    NA = B * NQA  # total AllToAlls
    SCA = GQA * QB // NCORES  # tokens per (core, a2a)
    CPQ = QB // SCA  # a2a chunks per q-block
    OCB = H // QB  # o_proj output column chunks
    WOPRE = min(3, OCB)  # wo ocs prefetched during phase 2
    ATG = min(8, HB)  # dblks per a2a-out load DMA
    scale = 1.0 / math.sqrt(P)

    xT = nc.declare_dram_parameter("xT", [H, BT], BF16, isOutput=False)
    wq_c = nc.declare_dram_parameter("wq_c", [H, DQ], BF16, isOutput=False)
    wk_c = nc.declare_dram_parameter("wk_c", [H, P], BF16, isOutput=False)
    wv_c = nc.declare_dram_parameter("wv_c", [H, P], BF16, isOutput=False)
    wo_full = nc.declare_dram_parameter("wo_full", [NH * P, H], BF16, isOutput=False)
    cos_t = nc.declare_dram_parameter("cos_t", [P, BT], F32, isOutput=False)
    sinx_t = nc.declare_dram_parameter("sinx_t", [P, BT], F32, isOutput=False)
    masks2_t = nc.declare_dram_parameter("masks2_t", [2, P, 2 * QB], BF16, isOutput=False)
    ones_t = nc.declare_dram_parameter("ones_t", [P, P], BF16, isOutput=False)
    y_c = nc.declare_dram_parameter("y_c", [NA * SCA, H], F32, isOutput=True)

    with tile.TileContext(nc) as tc:
        with (
            tc.tile_pool(name="dram", bufs=1, space="DRAM") as dram,
            tc.tile_pool(name="const", bufs=1) as constp,
            tc.tile_pool(name="persist", bufs=1) as persist,
        ):
            a2a_in = [
                dram.tile([NCORES * DQ, SCA], BF16, tag=f"a2ain{a}", name=f"a2ain{a}")
                for a in range(NA)
            ]
            a2a_out = [
                dram.tile([NCORES * DQ, SCA], BF16, tag=f"a2aout{a}", name=f"a2aout{a}")
                for a in range(NA)
            ]

            ones_sb = constp.tile([P, P], BF16, tag="ones")
            nc.sync.dma_start(out=ones_sb, in_=ones_t[:, :])
            mask_sb = [
                constp.tile([P, 2 * QB], BF16, tag=f"mask{o}", name=f"mask{o}")
                for o in range(2)
            ]
            for o in range(2):
                nc.sync.dma_start(out=mask_sb[o], in_=masks2_t[o])

            qT = [
                persist.tile([P, BT], BF16, tag=f"qT{i}", name=f"qT{i}")
                for i in range(NHC)
            ]
            kT = persist.tile([P, BT], BF16, tag="kT")
            vbig = persist.tile([P, BT], BF16, tag="vbig")

            # ---------------- phase 1: q/k/v projections + rope
            with (
                tc.tile_pool(name="xin", bufs=48) as xin_p,
                tc.tile_pool(name="wqkv", bufs=1) as w_p,
                tc.tile_pool(name="tabs", bufs=2) as tab_p,
                tc.tile_pool(name="ropetmp", bufs=4) as rt_p,
                tc.tile_pool(name="psq", bufs=1, space="PSUM") as psq_p,
                tc.tile_pool(name="psk", bufs=2, space="PSUM") as psk_p,
                tc.tile_pool(name="psv", bufs=2, space="PSUM") as psv_p,
            ):
                wq_sb = [w_p.tile([P, DQ], BF16, tag=f"wq{i}", name=f"wq{i}") for i in range(HB)]
                wk_sb = [w_p.tile([P, P], BF16, tag=f"wk{i}", name=f"wk{i}") for i in range(HB)]
                wv_sb = [w_p.tile([P, P], BF16, tag=f"wv{i}", name=f"wv{i}") for i in range(HB)]
                for hb in range(HB):
                    nc.sync.dma_start(out=wq_sb[hb], in_=wq_c[hb * P : (hb + 1) * P, :])
                    nc.sync.dma_start(out=wk_sb[hb], in_=wk_c[hb * P : (hb + 1) * P, :])
                    nc.sync.dma_start(out=wv_sb[hb], in_=wv_c[hb * P : (hb + 1) * P, :])

                def rope(dst, ps, cos_sb, sinx_sb):
                    """dst = ps*cos + shift64(ps)*sinx (all [128,QB])"""
                    t1 = rt_p.tile([P, QB], F32, tag="ropet1")
                    t2 = rt_p.tile([P, QB], F32, tag="ropet2")
                    nc.vector.tensor_tensor(t1, ps, cos_sb, mybir.AluOpType.mult)
                    h = P // 2
                    nc.vector.tensor_tensor(
                        t2[0:h], ps[h:P], sinx_sb[0:h], mybir.AluOpType.mult
                    )
                    nc.vector.tensor_tensor(
                        t2[h:P], ps[0:h], sinx_sb[h:P], mybir.AluOpType.mult
                    )
                    nc.vector.tensor_tensor(dst, t1, t2, mybir.AluOpType.add)

                dqgroups = [
                    tuple(range(g, min(g + 2, NHC))) for g in range(0, NHC, 2)
                ]

                for tb in range(TB):
                    t0 = tb * QB
                    cos_sb = tab_p.tile([P, QB], F32, tag="cos")
                    sinx_sb = tab_p.tile([P, QB], F32, tag="sinx")
                    nc.sync.dma_start(out=cos_sb, in_=cos_t[:, t0 : t0 + QB])
                    nc.sync.dma_start(out=sinx_sb, in_=sinx_t[:, t0 : t0 + QB])

                    xts = []
                    for hb in range(HB):
                        xi = xin_p.tile([P, QB], BF16, tag="xin")
                        nc.sync.dma_start(
                            out=xi, in_=xT[hb * P : (hb + 1) * P, t0 : t0 + QB]
                        )
                        xts.append(xi)

                    # chain groups: q-head pairs, then k; one v column-chain per
                    # group (v chains share a bank -> sequential among selves)
                    v_ps = psv_p.tile([P, QB], F32, tag="vps")
                    vchains = list(range(QB // P))
                    groups = [("q", grp) for grp in dqgroups] + [("k", None)]
                    while len(groups) < len(vchains):
                        groups.append(("", None))
                    for gi, (kind, grp) in enumerate(groups):
                        vi = vchains[gi] if gi < len(vchains) else None
                        q_ps = {}
                        if kind == "q":
                            q_ps = {
                                dq: psq_p.tile(
                                    [P, QB], F32, tag=f"qps{dq}", name=f"qps{dq}",
                                    bufs=(2 if NHC == 1 else 1),
                                )
                                for dq in grp
                            }
                        elif kind == "k":
                            k_ps = psk_p.tile([P, QB], F32, tag="kps")
                        for hb in range(HB):
                            if kind == "q":
                                for dq in grp:
                                    nc.tensor.matmul(
                                        q_ps[dq],
                                        wq_sb[hb][:, dq * P : (dq + 1) * P],
                                        xts[hb],
                                        start=(hb == 0),
                                        stop=(hb == HB - 1),
                                    )
                            elif kind == "k":
                                nc.tensor.matmul(
                                    k_ps,
                                    wk_sb[hb],
                                    xts[hb],
                                    start=(hb == 0),
                                    stop=(hb == HB - 1),
                                )
                            if vi is not None:
                                nc.tensor.matmul(
                                    v_ps[:, vi * P : (vi + 1) * P],
                                    xts[hb][:, vi * P : (vi + 1) * P],
                                    wv_sb[hb],
                                    start=(hb == 0),
                                    stop=(hb == HB - 1),
                                )
                        if kind == "q":
                            for dq in grp:
                                rope(qT[dq][:, t0 : t0 + QB], q_ps[dq], cos_sb, sinx_sb)
                        elif kind == "k":
                            rope(kT[:, t0 : t0 + QB], k_ps, cos_sb, sinx_sb)
                    nc.vector.tensor_copy(vbig[:, t0 : t0 + QB], v_ps)

            # ---------------- phases 2+3 share the wo pool (prefetch)
            with tc.tile_pool(name="wo", bufs=3 * HB) as wo_p:
                wo_sb = {}  # (oc, dblk) -> tile
                wo_pending = [(oc, dblk) for oc in range(WOPRE) for dblk in range(HB)]
                wo_ptr = [0]

                def emit_wo(n):
                    end = min(wo_ptr[0] + n, len(wo_pending))
                    for idx in range(wo_ptr[0], end):
                        oc, dblk = wo_pending[idx]
                        w = wo_p.tile([P, QB], BF16, tag="wo", name=f"wo{oc}_{dblk}")
                        nc.sync.dma_start(
                            out=w,
                            in_=wo_full[
                                dblk * P : (dblk + 1) * P, oc * QB : (oc + 1) * QB
                            ],
                        )
                        wo_sb[(oc, dblk)] = w
                    wo_ptr[0] = end

                # ---------------- phase 2: attention + split AllToAlls
                with (
                    tc.tile_pool(name="pP", bufs=3) as p_p,
                    tc.tile_pool(name="pT", bufs=2) as t_p,
                    tc.tile_pool(name="aout", bufs=4) as ao_p,
                    tc.tile_pool(name="rv", bufs=2) as rv_p,
                    tc.tile_pool(name="psS", bufs=2, space="PSUM") as pss_p,
                    tc.tile_pool(name="psO", bufs=2, space="PSUM") as pso_p,
                    tc.tile_pool(name="psL", bufs=1, space="PSUM") as psl_p,
                    tc.tile_pool(name="psR", bufs=1, space="PSUM") as psr_p,
                ):
                    def emit_norm(st):
                        """deferred normalization tail of a previous iteration"""
                        o_ps, l_ps, aidx, chunk0, h = st
                        rinv = rv_p.tile([1, QB], F32, tag="rinv")
                        nc.vector.reciprocal_approx_fast(out=rinv, in_=l_ps)
                        rinv_b = rv_p.tile([1, QB], BF16, tag="rinvb")
                        nc.vector.tensor_copy(rinv_b, rinv)
                        rb_ps = psr_p.tile([P, QB], F32, tag="rbps")
                        nc.tensor.matmul(
                            rb_ps, ones_sb[0:1, :], rinv_b, start=True, stop=True
                        )
                        rb_sb = rv_p.tile([P, QB], F32, tag="rbsb")
                        nc.scalar.copy(rb_sb, rb_ps)
                        attn_sb = ao_p.tile([P, QB], BF16, tag="attn")
                        nc.vector.tensor_tensor(
                            attn_sb, o_ps, rb_sb, mybir.AluOpType.mult
                        )
                        for cc in range(CPQ):
                            nc.sync.dma_start(
                                out=a2a_in[aidx][
                                    (chunk0 + cc) * DQ + h * P :
                                    (chunk0 + cc) * DQ + (h + 1) * P,
                                    :,
                                ],
                                in_=attn_sb[:, cc * SCA : (cc + 1) * SCA],
                            )

                    prev = None
                    for b in range(B):
                        for qq in range(NQA):
                            aidx = b * NQA + qq
                            for qb in range(qq * GQA, min((qq + 1) * GQA, QBB)):
                                for h in range(NHC):
                                    q0 = b * S + qb * QB
                                    npair = (qb + 1) * (QB // KB) // 2
                                    o_ps = pso_p.tile([P, QB], F32, tag="ops")
                                    t_sb = t_p.tile([P, 2 * QB], BF16, tag="T")
                                    for j in range(npair):
                                        if j == 1 and prev is not None:
                                            emit_norm(prev)
                                            prev = None
                                        k0 = b * S + j * 2 * KB
                                        s_ps = pss_p.tile([P, 2 * QB], F32, tag="sps")
                                        nc.tensor.matmul(
                                            s_ps[:, 0:QB],
                                            kT[:, k0 : k0 + KB],
                                            qT[h][:, q0 : q0 + QB],
                                            start=True,
                                            stop=True,
                                        )
                                        nc.tensor.matmul(
                                            s_ps[:, QB:],
                                            kT[:, k0 + KB : k0 + 2 * KB],
                                            qT[h][:, q0 : q0 + QB],
                                            start=True,
                                            stop=True,
                                        )
                                        p_sb = p_p.tile([P, 2 * QB], BF16, tag="P")
                                        nc.scalar.activation(
                                            p_sb,
                                            s_ps,
                                            mybir.ActivationFunctionType.Exp,
                                            scale=scale,
                                        )
                                        jd = j - qb * (QB // KB) // 2
                                        if jd >= 0:
                                            nc.vector.tensor_tensor(
                                                p_sb, p_sb, mask_sb[jd],
                                                mybir.AluOpType.mult,
                                            )
                                        if j == 0:
                                            nc.vector.tensor_copy(t_sb, p_sb)
                                        else:
                                            nc.vector.tensor_tensor(
                                                t_sb, t_sb, p_sb, mybir.AluOpType.add
                                            )
                                        nc.tensor.matmul(
                                            o_ps,
                                            vbig[:, k0 : k0 + P],
                                            p_sb[:, 0:QB],
                                            start=(j == 0),
                                            stop=False,
                                        )
                                        nc.tensor.matmul(
                                            o_ps,
                                            vbig[:, k0 + P : k0 + 2 * P],
                                            p_sb[:, QB:],
                                            start=False,
                                            stop=(j == npair - 1),
                                        )
                                    l_ps = psl_p.tile([1, QB], F32, tag="lps")
                                    nc.tensor.matmul(
                                        l_ps, ones_sb[:, 0:1], t_sb[:, 0:QB],
                                        start=True, stop=False,
                                    )
                                    nc.tensor.matmul(
                                        l_ps, ones_sb[:, 0:1], t_sb[:, QB:],
                                        start=False, stop=True,
                                    )
                                    prev = (o_ps, l_ps, aidx, (qb % GQA) * CPQ, h)
                                    emit_wo(3)
                            # flush before the collective so its inputs are emitted
                            if prev is not None:
                                emit_norm(prev)
                                prev = None
                            nc.gpsimd.collective_compute(
                                "AllToAll",
                                mybir.AluOpType.bypass,
                                replica_groups=[list(range(NCORES))],
                                ins=[a2a_in[aidx][:, :]],
                                outs=[a2a_out[aidx][:, :]],
                            )

                # ---------------- phase 3: row-parallel o_proj on own tokens
                with (
                    tc.tile_pool(name="aT", bufs=1) as at_p,
                    tc.tile_pool(name="yout", bufs=4) as y_p,
                    tc.tile_pool(name="psY", bufs=1, space="PSUM") as psy_p,
                ):
                    at_sb = []  # [a][g] tile [P, ATG*SCA]
                    for a in range(NA):
                        row = []
                        for g in range(HB // ATG):
                            t = at_p.tile(
                                [P, ATG * SCA], BF16, tag=f"aT{a}_{g}", name=f"aT{a}_{g}"
                            )
                            src = a2a_out[a][
                                g * ATG * P : (g + 1) * ATG * P, :
                            ].rearrange("(g p) s -> p g s", g=ATG)
                            dst = t.rearrange("p (g s) -> p g s", g=ATG)
                            nc.sync.dma_start(out=dst, in_=src)
                            row.append(t)
                        at_sb.append(row)

                    def at_slice(a, dblk):
                        g, j = dblk // ATG, dblk % ATG
                        return at_sb[a][g][:, j * SCA : (j + 1) * SCA]

                    for oc in range(OCB):
                        emit_wo(HB)  # no-op once pending prefetch is drained
                        if oc >= WOPRE:
                            for dblk in range(HB):
                                w = wo_p.tile([P, QB], BF16, tag="wo", name=f"wo{oc}_{dblk}")
                                nc.sync.dma_start(
                                    out=w,
                                    in_=wo_full[
                                        dblk * P : (dblk + 1) * P,
                                        oc * QB : (oc + 1) * QB,
                                    ],
                                )
                                wo_sb[(oc, dblk)] = w
                        for a0 in range(0, NA, 2):
                            pair = [a for a in (a0, a0 + 1) if a < NA]
                            y_ps = {
                                a: psy_p.tile(
                                    [SCA, QB], F32, tag=f"yps{a}", name=f"yps{a}"
                                )
                                for a in pair
                            }
                            for dblk in range(HB):
                                for a in pair:
                                    nc.tensor.matmul(
                                        y_ps[a],
                                        at_slice(a, dblk),
                                        wo_sb[(oc, dblk)],
                                        start=(dblk == 0),
                                        stop=(dblk == HB - 1),
                                    )
                            for ai, a in enumerate(pair):
                                y_sb = y_p.tile([SCA, QB], F32, tag="ysb")
                                if (a0 + ai) % 2 == 0:
                                    nc.scalar.copy(y_sb, y_ps[a])
                                else:
                                    nc.vector.tensor_copy(y_sb, y_ps[a])
                                # scalar-issued DMA: keeps the sync queue free of
                                # compute-gated stores so wo prefetch runs ahead
                                nc.scalar.dma_start(
                                    out=y_c[
                                        a * SCA : (a + 1) * SCA,
                                        oc * QB : (oc + 1) * QB,
                                    ],
                                    in_=y_sb,
                                )
    nc.finalize()
    return nc


def _prep_inputs(hidden_states, wq, wk, wv, wo, position_ids, B, S, H, NH, NKV):
    """Host-side: bf16 casts, x transpose, rope tables, paired causal masks,
    per-core weight slices."""
    BT = B * S
    NHC = NH // NCORES
    DQ = NHC * P

    bf = ml_dtypes.bfloat16
    x2d = np.asarray(hidden_states, dtype=np.float32).reshape(BT, H)
    xT = np.ascontiguousarray(x2d.T).astype(bf)
    wq_b, wk_b, wv_b, wo_b = (np.asarray(w).astype(bf) for w in (wq, wk, wv, wo))

    half = P // 2
    inv_freq = 1.0 / (10000.0 ** (np.arange(half, dtype=np.float64) / half))
    pos = np.asarray(position_ids).astype(np.float64).reshape(BT)
    ang = pos[None, :] * inv_freq[:, None]  # [64, BT]
    cos_t = np.concatenate([np.cos(ang), np.cos(ang)], 0).astype(np.float32)
    sinx_t = np.concatenate([-np.sin(ang), np.sin(ang)], 0).astype(np.float32)

    # paired diagonal-block causal masks over [128 k, 2*QB (2 k-blocks x q)]
    kk = np.arange(KB)[None, :, None]
    qq = np.arange(QB)[None, None, :]
    masks2 = np.zeros((2, P, 2 * QB), dtype=bf)
    for jd in range(2):
        for hhalf in range(2):
            krel = (2 * jd + hhalf) * KB + kk
            masks2[jd : jd + 1, :, hhalf * QB : (hhalf + 1) * QB] = (krel <= qq).astype(bf)
    ones_m = np.ones((P, P), dtype=bf)

    in_maps = []
    for c in range(NCORES):
        in_maps.append(
            {
                "xT": xT,
                "wq_c": np.ascontiguousarray(wq_b[:, c * DQ : (c + 1) * DQ]),
                "wk_c": np.ascontiguousarray(wk_b[:, c * P : (c + 1) * P]),
                "wv_c": np.ascontiguousarray(wv_b[:, c * P : (c + 1) * P]),
                "wo_full": wo_b,
                "cos_t": cos_t,
                "sinx_t": sinx_t,
                "masks2_t": masks2,
                "ones_t": ones_m,
            }
        )
    return in_maps


def run(hidden_states, wq, wk, wv, wo, position_ids, B, S, H, NH, NKV, trace=False):
    key = (B, S, H, NH, NKV)
    if key not in _CACHE:
        _CACHE[key] = build_program(B, S, H, NH, NKV)
    nc = _CACHE[key]
    in_maps = _prep_inputs(
        hidden_states, wq, wk, wv, wo, position_ids, B, S, H, NH, NKV
    )
    res = run_bass_kernel_spmd(nc, in_maps, core_ids=list(range(NCORES)), trace=trace)
    QBB = S // QB
    GQA = min(2, QBB)
    NQA = (QBB + GQA - 1) // GQA
    NA = B * NQA
    SCA = GQA * QB // NCORES
    out = np.empty((B, S, NH * P), dtype=np.float32)
    for c in range(NCORES):
        yc = res.results[c]["y_c"]
        for a in range(NA):
            b, qq = a // NQA, a % NQA
            tok0 = qq * GQA * QB + c * SCA
            out[b, tok0 : tok0 + SCA, :] = yc[a * SCA : (a + 1) * SCA, :]
    return (out, res) if trace else (out, None)


def kernel(hidden_states, wq, wk, wv, wo, position_ids):
    out, _ = run(
        hidden_states, wq, wk, wv, wo, position_ids, 2, 2048, 4096, 32, 8
    )
    return out
